# revision 15
# baseline (speedup 1.0000x reference)
"""Trainium2 Bass kernel for nn_Adapter2 (dense_cnn).

Strategy (8 NeuronCores, data-parallel over the clip dim B, zero collectives):
  Each core takes 32 of the 256 N-columns (2 clips x 16 frames). Host-side we
  pre-transpose x to channel-major xT [768, 197*32] per core (bf16), with a
  per-tile b-major token order (b, l, t), so every DMA is contiguous and the
  contraction dim lands on partitions.

  Per core the whole module collapses into two matmul stages:
    A: y[576, cols] = [fc1_w | mlp_in_w | off_fc1_w]^T @ xT      (bf16, PSUM f32)
    elementwise middle (reads PSUM directly):
      - temporal depthwise conv (3 taps) on the fc1 block  -> z rows 0..192
      - quickGELU via ACT Silu(1.702*y+1.702*b) (1/1.702 folded into W2) -> rows 192..384
      - temporal diff into a zero-guarded padded buffer, then 3x3 spatial
        depthwise conv as 9 accumulating shifted-AP taps -> rows 384..576
      - two aux rows (all-ones, l>=1 indicator) carry all output-side biases
    B: outT[768, cols] = Wcat2[578, 768]^T @ z   (single PSUM accumulation sums
       all three branches + biases), evict bf16, DMA out.

  Token tiles follow spatial h-rows (14 l-rows = one row of the 14x14 grid),
  so the spatial conv taps are affine shifted APs into the padded buffer.
  The 64-partition offset chunk (ca 128..192) is b-packed onto 128 partitions
  (contiguous SBUF->SBUF copies thanks to the b-major order) so its conv taps
  run at half the free size.
"""
import sys

if "/opt/trn_rl_repo" not in sys.path:
    sys.path.insert(0, "/opt/trn_rl_repo")

import numpy as np
import ml_dtypes

import concourse.bass as bass
import concourse.mybir as mybir
from concourse.tile import TileContext
from concourse import bass_utils, bacc

F32 = mybir.dt.float32
BF16 = mybir.dt.bfloat16
AF = mybir.ActivationFunctionType
OP = mybir.AluOpType

C = 768
CA = 192
L = 197
T = 16
NCORES = 8
NL = 32                      # N-columns per core (2 clips x 16 frames)
NCOLS = L * NL               # 6304
HGRID = 14
PADW = 16                    # padded grid row stride (w in -1..14)
PPAD_COLS = 2 * 16 * PADW * T   # (b) x (h in -1..14) x (w in -1..14) x t
PPADB_COLS = 16 * PADW * T      # b-packed: (h) x (w) x t = 4096

# token tiles: tile 0 = l 0..14 (CLS + h-row 0), tiles 1..13 = h-rows 1..13
TILES = [(0, 15)] + [(1 + 14 * k, 14) for k in range(1, 14)]
TILE_C0 = [0]
for _, _nl in TILES:
    TILE_C0.append(TILE_C0[-1] + _nl * NL)

NSCAL = 52
TAPS = [(dh, dw) for dh in (-1, 0, 1) for dw in (-1, 0, 1)]


def _pack_scalars(conv_w, conv_b, fc1_b, mlp_in_b, off_fc1_b, off_conv_w, off_conv_b):
    """Per-partition scalar pack [128, NSCAL] f32, partition-aligned per op."""
    s = np.zeros((128, NSCAL), np.float32)
    wsp = off_conv_w[:, 0, 0, :, :]  # (CA, 3, 3)
    w0, w1, w2 = conv_w[:, 0, 0], conv_w[:, 0, 1], conv_w[:, 0, 2]
    wsum_b = (w0 + w1 + w2) * fc1_b + conv_b
    # conv_t chunk A (ca 0..128, rows 0..128)
    s[:, 0] = w0[:128]; s[:, 1] = w1[:128]; s[:, 2] = w2[:128]
    s[:, 3] = wsum_b[:128]
    s[:, 4] = (-w0 * fc1_b)[:128]; s[:, 5] = (-w2 * fc1_b)[:128]
    # conv_t chunk B (ca 128..192, rows 0..64)
    s[:64, 6] = w0[128:]; s[:64, 7] = w1[128:]; s[:64, 8] = w2[128:]
    s[:64, 9] = wsum_b[128:]
    s[:64, 10] = (-w0 * fc1_b)[128:]; s[:64, 11] = (-w2 * fc1_b)[128:]
    # gelu biases (scaled by 1.702; Silu(1.702 g + 1.702 b) = 1.702 qgelu(g+b))
    s[64:, 12] = 1.702 * mlp_in_b[0:64]
    s[:, 13] = 1.702 * mlp_in_b[64:192]
    # off_fc1 bias (chunk A)
    s[:, 14] = off_fc1_b[:128]
    # spatial taps + conv bias (chunk A)
    for t_i, (dh, dw) in enumerate(TAPS):
        s[:, 16 + t_i] = wsp[:128, dh + 1, dw + 1]
    s[:, 34] = off_conv_b[:128]
    s[64, 36] = 1.0   # aux1 keep
    s[65, 36] = 0.0   # aux2 zeroed on CLS cols
    # b-packed duplicates for the offset-branch chunk B (ca 128..192 on both
    # partition halves: rows 0:64 = clip b=0, rows 64:128 = clip b=1)
    dup = lambda a: np.concatenate([a[128:], a[128:]])
    s[:, 37] = dup(off_fc1_b)
    for t_i, (dh, dw) in enumerate(TAPS):
        s[:, 38 + t_i] = dup(wsp[:, dh + 1, dw + 1])
    s[:, 47] = dup(off_conv_b)
    # conv_t t-edge center biases: t0 = (w1+w2)b+cb, t15 = (w0+w1)b+cb
    t0b = (w1 + w2) * fc1_b + conv_b
    t15b = (w0 + w1) * fc1_b + conv_b
    s[:, 48] = t0b[:128]; s[:64, 49] = t0b[128:]
    s[:, 50] = t15b[:128]; s[:64, 51] = t15b[128:]
    return s


def build_kernel():
    nc = bacc.Bacc("TRN2", target_bir_lowering=False, debug=False,
                   num_devices=NCORES)
    xt_d = nc.declare_dram_parameter("xt", [C, NCOLS], BF16, isOutput=False)
    w1_d = nc.declare_dram_parameter("w1", [C, 576], BF16, isOutput=False)
    w2_d = nc.declare_dram_parameter("w2", [578, C], BF16, isOutput=False)
    sc_d = nc.declare_dram_parameter("scal", [128, NSCAL], F32, isOutput=False)
    out_d = nc.declare_dram_parameter("out", [C, NCOLS], BF16, isOutput=True)

    # z chunk partition sizes (576 rows + 2 aux packed as 4x128 + 66)
    ZP = [128, 128, 128, 128, 66]

    with TileContext(nc) as tc:
        with (
            tc.tile_pool(name="const", bufs=1) as cpool,
            tc.tile_pool(name="xin", bufs=1) as xpool,
            tc.tile_pool(name="z", bufs=3) as zpool,
            tc.tile_pool(name="osb", bufs=4) as opool_sb,
            tc.tile_pool(name="ypsum", bufs=1, space="PSUM") as ypool,
            tc.tile_pool(name="opsum", bufs=3, space="PSUM") as opool,
        ):
            # --- constants; interleave tile-0 x loads with the weights so
            # the first A matmuls start ASAP ---
            scal = cpool.tile([128, NSCAL], F32, name="scal")
            nc.sync.dma_start(out=scal[:], in_=sc_d[:])
            xt_sb = [[None] * 6 for _ in TILES]
            w1_sb = []
            w00 = TILES[0][1] * NL
            for i in range(6):
                xt = xpool.tile([128, w00], BF16, name=f"xt_0_{i}")
                nc.sync.dma_start(out=xt[:], in_=xt_d[i * 128:(i + 1) * 128,
                                                      0:w00])
                xt_sb[0][i] = xt
                t = cpool.tile([128, 576], BF16, name=f"w1_{i}")
                nc.sync.dma_start(out=t[:], in_=w1_d[i * 128:(i + 1) * 128, :])
                w1_sb.append(t)
            w2_sb = []
            for kk in range(5):
                p0 = kk * 128
                pn = ZP[kk]
                t = cpool.tile([pn, C], BF16, name=f"w2_{kk}")
                nc.sync.dma_start(out=t[:], in_=w2_d[p0:p0 + pn, :])
                w2_sb.append(t)

            ppad_a = cpool.tile([128, PPAD_COLS], BF16, name="ppad_a")
            ppad_b = cpool.tile([128, PPADB_COLS], BF16, name="ppad_b")
            nc.gpsimd.memset(ppad_a[:], 0.0)
            nc.gpsimd.memset(ppad_b[:], 0.0)

            # --- second tile up-front; the rest are prefetched inside the
            # main loop so per-tile DMAs aren't queued behind them on the
            # SP sequencer ---
            def load_xt(k):
                c0, w = TILE_C0[k], TILES[k][1] * NL
                for i in range(6):
                    t = xpool.tile([128, w], BF16, name=f"xt_{k}_{i}")
                    nc.sync.dma_start(out=t[:], in_=xt_d[i * 128:(i + 1) * 128,
                                                         c0:c0 + w])
                    xt_sb[k][i] = t
            load_xt(1)

            def col(j, rows=128):
                return scal[0:rows, j:j + 1]

            def colr(r0, r1, j):
                return scal[r0:r1, j:j + 1]

            # padded-grid views: ppad_a [p, b, lpad, t], ppad_b [p, lpad, t]
            pav = ppad_a[0:128, :].rearrange("p (b l t) -> p b l t", b=2, t=T)
            pbv = ppad_b[0:128, :].rearrange("p (l t) -> p l t", t=T)

            z_tiles = [None] * 14   # per tile: [zc0..zc4]

            def emit_A_and_middle(k):
                nl = TILES[k][1]
                w = nl * NL
                loff = 1 if k == 0 else 0   # skip CLS l-row in tile 0
                # ---- matmul A: y chunks = Wcat1 col blocks ----
                ys = []
                for m in range(5):
                    m0 = m * 128
                    mw = min(128, 576 - m0)
                    yt = ypool.tile([mw, w], F32, name=f"y{m}")
                    for i in range(6):
                        nc.tensor.matmul(yt[:, :], w1_sb[i][:, m0:m0 + mw],
                                         xt_sb[k][i][:, :],
                                         start=(i == 0), stop=(i == 5))
                    ys.append(yt)

                zc = [zpool.tile([ZP[j], 480], BF16, name=f"zc{j}")
                      for j in range(5)]
                z_tiles[k] = zc

                # views [p, b, l, t]
                def v(ap, p0, p1):
                    return ap[p0:p1, 0:w].rearrange(
                        "p (b l t) -> p b l t", b=2, t=T)

                # ---- conv_t (main branch): center tap split over t-ranges
                # with pre-folded edge biases (ACT), then two accumulating
                # shifted taps (DVE) ----
                for (ysrc, yp0, yp1, zdst, cw0, cw1, cw2, cs2, cf0,
                     cf15, rows) in (
                        (ys[0], 0, 128, zc[0], 0, 1, 2, 3, 4, 5, 128),
                        (ys[1], 0, 64, zc[1], 6, 7, 8, 9, 10, 11, 64)):
                    yv = v(ysrc, yp0, yp1)
                    zv = v(zdst, yp0, yp1)
                    # full-range center (interior bias), then t-edge bias
                    # corrections on size-1 t-slices (both <=3D after opt)
                    nc.scalar.activation(
                        zv, yv, AF.Identity,
                        bias=col(cs2, rows), scale=col(cw1, rows))
                    nc.scalar.activation(
                        zv[:, :, :, 0:1], zv[:, :, :, 0:1], AF.Identity,
                        bias=col(cf0, rows), scale=1.0)
                    nc.scalar.activation(
                        zv[:, :, :, T - 1:T], zv[:, :, :, T - 1:T], AF.Identity,
                        bias=col(cf15, rows), scale=1.0)
                    # shifted taps per clip (3D APs); grouped left-left /
                    # right-right so chained RAW deps are 2 ops apart
                    for b in (0, 1):
                        nc.vector.scalar_tensor_tensor(
                            out=zv[:, b, :, 1:], in0=yv[:, b, :, :T - 1],
                            scalar=col(cw0, rows), in1=zv[:, b, :, 1:],
                            op0=OP.mult, op1=OP.add)
                    for b in (0, 1):
                        nc.vector.scalar_tensor_tensor(
                            out=zv[:, b, :, :T - 1], in0=yv[:, b, :, 1:],
                            scalar=col(cw2, rows), in1=zv[:, b, :, :T - 1],
                            op0=OP.mult, op1=OP.add)

                # ---- quickGELU branch (ACT) ----
                nc.scalar.activation(zc[1][64:128, :w], ys[1][64:128, :],
                                     AF.Silu, bias=colr(64, 128, 12),
                                     scale=1.702)
                nc.scalar.activation(zc[2][0:128, :w], ys[2][0:128, :],
                                     AF.Silu, bias=colr(0, 128, 13),
                                     scale=1.702)

                # ---- temporal diff -> padded buffers (h-row k) ----
                lp0 = (k + 1) * PADW + 1
                yo_a = zpool.tile([128, 480], BF16, name="yo_a")
                yo_b = zpool.tile([64, 480], BF16, name="yo_b")
                nc.scalar.activation(yo_a[:, :w], ys[3][:, :], AF.Copy)
                nc.scalar.activation(yo_b[:, :w], ys[4][:, :], AF.Copy)
                # chunk A (ca 0..128): [p, b, l, t] -> ppad_a[p, b, lpad, t]
                yv = v(yo_a, 0, 128)[:, :, loff:, :]
                pv = pav[:, :, lp0:lp0 + HGRID, :]
                for b in (0, 1):
                    nc.vector.scalar_tensor_tensor(
                        out=pv[:, b, :, 1:], in0=yv[:, b, :, 1:],
                        scalar=col(14, 128), in1=yv[:, b, :, :T - 1],
                        op0=OP.add, op1=OP.subtract)
                nc.scalar.activation(
                    pv[:, :, :, 0:1], yv[:, :, :, 0:1], AF.Identity,
                    bias=col(14, 128), scale=0.0)
                # chunk B: pack the two clips across partition halves
                # (contiguous b-blocks thanks to the b-major token order)
                yb2 = zpool.tile([128, 224], BF16, name="yb2")
                yo_b4 = v(yo_b, 0, 64)[:, :, loff:, :]
                nc.sync.dma_start(
                    out=yb2[0:64, :].rearrange("p (o l t) -> p o l t",
                                               o=1, t=T),
                    in_=yo_b4[:, 0:1])
                nc.sync.dma_start(
                    out=yb2[64:128, :].rearrange("p (o l t) -> p o l t",
                                                 o=1, t=T),
                    in_=yo_b4[:, 1:2])
                yv2 = yb2[0:128, :].rearrange("p (l t) -> p l t", t=T)
                pv2 = pbv[:, lp0:lp0 + HGRID, :]
                nc.vector.scalar_tensor_tensor(
                    out=pv2[:, :, 1:], in0=yv2[:, :, 1:],
                    scalar=col(37, 128), in1=yv2[:, :, :T - 1],
                    op0=OP.add, op1=OP.subtract)
                nc.scalar.activation(
                    pv2[:, :, 0:1], yv2[:, :, 0:1], AF.Identity,
                    bias=col(37, 128), scale=0.0)

                # ---- aux rows ----
                nc.gpsimd.memset(zc[4][64:66, :w], 1.0)
                if k == 0:
                    # zero aux2 + the offset-branch z rows on the CLS cols
                    cls4 = zc[4][64:66, 0:w].rearrange(
                        "p (b l t) -> p b l t", b=2, t=T)[:, :, 0:1, :]
                    nc.vector.tensor_scalar(
                        out=cls4, in0=cls4,
                        scalar1=colr(64, 66, 36), scalar2=None, op0=OP.mult)
                    nc.gpsimd.memset(zc[3][0:128, 0:w].rearrange(
                        "p (b l t) -> p b l t", b=2, t=T)[:, :, 0:1, :], 0.0)
                    nc.gpsimd.memset(zc[4][0:64, 0:w].rearrange(
                        "p (b l t) -> p b l t", b=2, t=T)[:, :, 0:1, :], 0.0)

            def emit_spconv_B_out(j):
                nl = TILES[j][1]
                w = nl * NL
                c0 = TILE_C0[j]
                loff = 1 if j == 0 else 0
                lp0 = (j + 1) * PADW + 1
                zc = z_tiles[j]
                # ---- spatial conv: 9 shifted taps from the padded buffers
                # (chunk A full-partition; chunk B b-packed, then unpacked
                # into the z layout via two contiguous DMAs) ----
                zv = zc[3][0:128, 0:w].rearrange(
                    "p (b l t) -> p b l t", b=2, t=T)[:, :, loff:, :]
                qb2 = zpool.tile([128, 224], BF16, name="qb2")
                qv = qb2[0:128, :].rearrange("p (l t) -> p l t", t=T)
                # interleave the two accumulation chains so consecutive DVE
                # ops never have an adjacent RAW dependency
                for t_i, (dh, dw) in enumerate(TAPS):
                    sl = lp0 + dh * PADW + dw
                    pv = pav[:, :, sl:sl + HGRID, :]
                    pv2 = pbv[:, sl:sl + HGRID, :]
                    if t_i == 0:
                        nc.scalar.activation(
                            zv, pv, AF.Identity,
                            bias=col(34, 128), scale=col(16 + t_i, 128))
                        nc.scalar.activation(
                            qv, pv2, AF.Identity,
                            bias=col(47, 128), scale=col(38 + t_i, 128))
                    else:
                        nc.vector.scalar_tensor_tensor(
                            out=zv, in0=pv, scalar=col(16 + t_i, 128),
                            in1=zv, op0=OP.mult, op1=OP.add)
                        nc.vector.scalar_tensor_tensor(
                            out=qv, in0=pv2, scalar=col(38 + t_i, 128),
                            in1=qv, op0=OP.mult, op1=OP.add)
                zb = zc[4][0:64, 0:w].rearrange(
                    "p (b l t) -> p b l t", b=2, t=T)[:, :, loff:, :]
                qb4 = qb2.rearrange("p (o l t) -> p o l t", o=1, t=T)
                nc.sync.dma_start(out=zb[:, 0:1], in_=qb4[0:64])
                nc.sync.dma_start(out=zb[:, 1:2], in_=qb4[64:128])

                # ---- matmul B + eviction + store ----
                for m in range(6):
                    m0 = m * 128
                    ot = opool.tile([128, w], F32, name="ops")
                    for kk in range(5):
                        nc.tensor.matmul(ot[:, :], w2_sb[kk][:, m0:m0 + 128],
                                         zc[kk][:, :w],
                                         start=(kk == 0), stop=(kk == 4))
                    osb = opool_sb.tile([128, w], BF16, name="osb")
                    nc.scalar.activation(osb[:, :], ot[:, :], AF.Copy)
                    nc.sync.dma_start(out=out_d[m0:m0 + 128, c0:c0 + w],
                                      in_=osb[:, :])

            for k in range(14):
                if k + 2 <= 13:
                    load_xt(k + 2)
                emit_A_and_middle(k)
                if k >= 2:
                    emit_spconv_B_out(k - 2)
            emit_spconv_B_out(12)
            emit_spconv_B_out(13)

    nc.compile()
    return nc


_cached = {}


def _get_kernel():
    if "nc" not in _cached:
        _cached["nc"] = build_kernel()
    return _cached["nc"]


def _host_xt(x):
    """x (L, 256, C) f32 -> (8, C, NCOLS) bf16, per-tile b-major token order."""
    bf = ml_dtypes.bfloat16
    out = np.empty((NCORES, C, NCOLS), bf)
    x5 = x.reshape(L, NCORES, 2, T, C)
    for k, (l0, nl) in enumerate(TILES):
        blk = x5[l0:l0 + nl]                      # (nl, 8, 2, T, C)
        blk = blk.transpose(1, 4, 2, 0, 3)        # (8, C, 2, nl, T)
        out[:, :, TILE_C0[k]:TILE_C0[k + 1]] = (
            blk.reshape(NCORES, C, nl * NL).astype(bf))
    return out


def _host_out(outT):
    """outT (8, C, NCOLS) -> out (L, 256, C) f32."""
    out = np.empty((L, NCORES * NL, C), np.float32)
    for k, (l0, nl) in enumerate(TILES):
        blk = outT[:, :, TILE_C0[k]:TILE_C0[k + 1]].astype(np.float32)
        blk = blk.reshape(NCORES, C, 2, nl, T)    # (8, C, 2, nl, T)
        blk = blk.transpose(3, 0, 2, 4, 1)        # (nl, 8, 2, T, C)
        out[l0:l0 + nl] = blk.reshape(nl, NCORES * NL, C)
    return out


def kernel(x, T, fc1_w, fc1_b, conv_w, conv_b, fc2_w, fc2_b,
           off_fc1_w, off_fc1_b, off_conv_w, off_conv_b, off_fc2_w, off_fc2_b,
           mlp_in_w, mlp_in_b, mlp_out_w, mlp_out_b):
    bf = ml_dtypes.bfloat16
    x = np.asarray(x, np.float32)
    to_np = lambda a: np.asarray(a, np.float32)
    (fc1_w, fc1_b, conv_w, conv_b, fc2_w, fc2_b, off_fc1_w, off_fc1_b,
     off_conv_w, off_conv_b, off_fc2_w, off_fc2_b, mlp_in_w, mlp_in_b,
     mlp_out_w, mlp_out_b) = map(to_np, (
        fc1_w, fc1_b, conv_w, conv_b, fc2_w, fc2_b, off_fc1_w, off_fc1_b,
        off_conv_w, off_conv_b, off_fc2_w, off_fc2_b, mlp_in_w, mlp_in_b,
        mlp_out_w, mlp_out_b))

    xt = _host_xt(x)

    w1 = np.concatenate([fc1_w, mlp_in_w, off_fc1_w], axis=1).astype(bf)
    w2 = np.concatenate([
        fc2_w,
        mlp_out_w / 1.702,
        off_fc2_w,
        (fc2_b + mlp_out_b)[None, :],
        off_fc2_b[None, :],
    ], axis=0).astype(bf)
    scal = _pack_scalars(conv_w, conv_b, fc1_b, mlp_in_b, off_fc1_b,
                         off_conv_w, off_conv_b)

    nc = _get_kernel()
    in_maps = [{"xt": xt[i], "w1": w1, "w2": w2, "scal": scal}
               for i in range(NCORES)]
    res = bass_utils.run_bass_kernel_spmd(nc, in_maps,
                                          core_ids=list(range(NCORES)))
    _cached["last_result"] = res

    outT = np.stack([np.asarray(res.results[i]["out"]) for i in range(NCORES)])
    return np.ascontiguousarray(_host_out(outT))


# revision 16
# speedup vs baseline: 1.1264x; 1.1264x over previous
"""Trainium2 Bass kernel for nn_Adapter2 (dense_cnn).

Strategy (8 NeuronCores, data-parallel over the clip dim B, zero collectives):
  Each core takes 32 of the 256 N-columns (2 clips x 16 frames). Host-side we
  pre-transpose x to channel-major xT [768, 197*32] per core (bf16), with a
  per-tile b-major token order (b, l, t), so every DMA is contiguous and the
  contraction dim lands on partitions.

  Per core the whole module collapses into two matmul stages:
    A: y[576, cols] = [fc1_w | mlp_in_w | off_fc1_w]^T @ xT      (bf16, PSUM f32)
    elementwise middle (reads PSUM directly):
      - temporal depthwise conv (3 taps) on the fc1 block  -> z rows 0..192
      - quickGELU via ACT Silu(1.702*y+1.702*b) (1/1.702 folded into W2) -> rows 192..384
      - temporal diff into a zero-guarded padded buffer, then 3x3 spatial
        depthwise conv as 9 accumulating shifted-AP taps -> rows 384..576
      - two aux rows (all-ones, l>=1 indicator) carry all output-side biases
    B: outT[768, cols] = Wcat2[578, 768]^T @ z   (single PSUM accumulation sums
       all three branches + biases), evict bf16, DMA out.

  Token tiles follow spatial h-rows (14 l-rows = one row of the 14x14 grid),
  so the spatial conv taps are affine shifted APs into the padded buffer.
  The 64-partition offset chunk (ca 128..192) is b-packed onto 128 partitions
  (contiguous SBUF->SBUF copies thanks to the b-major order) so its conv taps
  run at half the free size.
"""
import sys

if "/opt/trn_rl_repo" not in sys.path:
    sys.path.insert(0, "/opt/trn_rl_repo")

import numpy as np
import ml_dtypes

import concourse.bass as bass
import concourse.mybir as mybir
from concourse.tile import TileContext
from concourse import bass_utils, bacc

F32 = mybir.dt.float32
BF16 = mybir.dt.bfloat16
AF = mybir.ActivationFunctionType
OP = mybir.AluOpType

C = 768
CA = 192
L = 197
T = 16
NCORES = 8
NL = 32                      # N-columns per core (2 clips x 16 frames)
NCOLS = L * NL               # 6304
HGRID = 14
PADW = 16                    # padded grid row stride (w in -1..14)
PPAD_COLS = 2 * 16 * PADW * T   # (b) x (h in -1..14) x (w in -1..14) x t
PPADB_COLS = 16 * PADW * T      # b-packed: (h) x (w) x t = 4096

# token tiles: tile 0 = l 0..14 (CLS + h-row 0), tiles 1..13 = h-rows 1..13
TILES = [(0, 15)] + [(1 + 14 * k, 14) for k in range(1, 14)]
TILE_C0 = [0]
for _, _nl in TILES:
    TILE_C0.append(TILE_C0[-1] + _nl * NL)

NSCAL = 52
TAPS = [(dh, dw) for dh in (-1, 0, 1) for dw in (-1, 0, 1)]


def _pack_scalars(conv_w, conv_b, fc1_b, mlp_in_b, off_fc1_b, off_conv_w, off_conv_b):
    """Per-partition scalar pack [128, NSCAL] f32, partition-aligned per op."""
    s = np.zeros((128, NSCAL), np.float32)
    wsp = off_conv_w[:, 0, 0, :, :]  # (CA, 3, 3)
    w0, w1, w2 = conv_w[:, 0, 0], conv_w[:, 0, 1], conv_w[:, 0, 2]
    wsum_b = (w0 + w1 + w2) * fc1_b + conv_b
    # conv_t chunk A (ca 0..128, rows 0..128)
    s[:, 0] = w0[:128]; s[:, 1] = w1[:128]; s[:, 2] = w2[:128]
    s[:, 3] = wsum_b[:128]
    s[:, 4] = (-w0 * fc1_b)[:128]; s[:, 5] = (-w2 * fc1_b)[:128]
    # conv_t chunk B (ca 128..192, rows 0..64)
    s[:64, 6] = w0[128:]; s[:64, 7] = w1[128:]; s[:64, 8] = w2[128:]
    s[:64, 9] = wsum_b[128:]
    s[:64, 10] = (-w0 * fc1_b)[128:]; s[:64, 11] = (-w2 * fc1_b)[128:]
    # gelu biases (scaled by 1.702; Silu(1.702 g + 1.702 b) = 1.702 qgelu(g+b))
    s[64:, 12] = 1.702 * mlp_in_b[0:64]
    s[:, 13] = 1.702 * mlp_in_b[64:192]
    # off_fc1 bias (chunk A)
    s[:, 14] = off_fc1_b[:128]
    # spatial taps + conv bias (chunk A)
    for t_i, (dh, dw) in enumerate(TAPS):
        s[:, 16 + t_i] = wsp[:128, dh + 1, dw + 1]
    s[:, 34] = off_conv_b[:128]
    s[64, 36] = 1.0   # aux1 keep
    s[65, 36] = 0.0   # aux2 zeroed on CLS cols
    # b-packed duplicates for the offset-branch chunk B (ca 128..192 on both
    # partition halves: rows 0:64 = clip b=0, rows 64:128 = clip b=1)
    dup = lambda a: np.concatenate([a[128:], a[128:]])
    s[:, 37] = dup(off_fc1_b)
    for t_i, (dh, dw) in enumerate(TAPS):
        s[:, 38 + t_i] = dup(wsp[:, dh + 1, dw + 1])
    s[:, 47] = dup(off_conv_b)
    # conv_t t-edge center biases: t0 = (w1+w2)b+cb, t15 = (w0+w1)b+cb
    t0b = (w1 + w2) * fc1_b + conv_b
    t15b = (w0 + w1) * fc1_b + conv_b
    s[:, 48] = t0b[:128]; s[:64, 49] = t0b[128:]
    s[:, 50] = t15b[:128]; s[:64, 51] = t15b[128:]
    return s


def build_kernel():
    nc = bacc.Bacc("TRN2", target_bir_lowering=False, debug=False,
                   num_devices=NCORES)
    xt_d = nc.declare_dram_parameter("xt", [C, NCOLS], BF16, isOutput=False)
    w1_d = nc.declare_dram_parameter("w1", [C, 576], BF16, isOutput=False)
    w2_d = nc.declare_dram_parameter("w2", [578, C], BF16, isOutput=False)
    sc_d = nc.declare_dram_parameter("scal", [128, NSCAL], F32, isOutput=False)
    out_d = nc.declare_dram_parameter("out", [C, NCOLS], BF16, isOutput=True)

    # z chunk partition sizes (576 rows + 2 aux packed as 4x128 + 66)
    ZP = [128, 128, 128, 128, 66]

    with TileContext(nc) as tc:
        with (
            tc.tile_pool(name="const", bufs=1) as cpool,
            tc.tile_pool(name="xin", bufs=1) as xpool,
            tc.tile_pool(name="z", bufs=3) as zpool,
            tc.tile_pool(name="osb", bufs=4) as opool_sb,
            tc.tile_pool(name="ypsum", bufs=1, space="PSUM") as ypool,
            tc.tile_pool(name="opsum", bufs=3, space="PSUM") as opool,
        ):
            # --- constants; interleave tile-0 x loads with the weights so
            # the first A matmuls start ASAP ---
            scal = cpool.tile([128, NSCAL], F32, name="scal")
            nc.sync.dma_start(out=scal[:], in_=sc_d[:])
            xt_sb = [[None] * 6 for _ in TILES]
            w1_sb = []
            w00 = TILES[0][1] * NL
            for i in range(6):
                xt = xpool.tile([128, w00], BF16, name=f"xt_0_{i}")
                nc.sync.dma_start(out=xt[:], in_=xt_d[i * 128:(i + 1) * 128,
                                                      0:w00])
                xt_sb[0][i] = xt
                t = cpool.tile([128, 576], BF16, name=f"w1_{i}")
                nc.sync.dma_start(out=t[:], in_=w1_d[i * 128:(i + 1) * 128, :])
                w1_sb.append(t)
            w2_sb = []
            for kk in range(5):
                p0 = kk * 128
                pn = ZP[kk]
                t = cpool.tile([pn, C], BF16, name=f"w2_{kk}")
                nc.sync.dma_start(out=t[:], in_=w2_d[p0:p0 + pn, :])
                w2_sb.append(t)

            ppad_a = cpool.tile([128, PPAD_COLS], BF16, name="ppad_a")
            ppad_b = cpool.tile([128, PPADB_COLS], BF16, name="ppad_b")
            nc.gpsimd.memset(ppad_a[:], 0.0)
            nc.gpsimd.memset(ppad_b[:], 0.0)

            # --- second tile up-front; the rest are prefetched inside the
            # main loop so per-tile DMAs aren't queued behind them on the
            # SP sequencer ---
            def load_xt(k):
                c0, w = TILE_C0[k], TILES[k][1] * NL
                for i in range(6):
                    t = xpool.tile([128, w], BF16, name=f"xt_{k}_{i}")
                    nc.sync.dma_start(out=t[:], in_=xt_d[i * 128:(i + 1) * 128,
                                                         c0:c0 + w])
                    xt_sb[k][i] = t
            load_xt(1)
            load_xt(2)

            def col(j, rows=128):
                return scal[0:rows, j:j + 1]

            def colr(r0, r1, j):
                return scal[r0:r1, j:j + 1]

            # padded-grid views: ppad_a [p, b, lpad, t], ppad_b [p, lpad, t]
            pav = ppad_a[0:128, :].rearrange("p (b l t) -> p b l t", b=2, t=T)
            pbv = ppad_b[0:128, :].rearrange("p (l t) -> p l t", t=T)

            z_tiles = [None] * 14   # per tile: [zc0..zc4]

            def emit_A_and_middle(k):
                nl = TILES[k][1]
                w = nl * NL
                loff = 1 if k == 0 else 0   # skip CLS l-row in tile 0
                # ---- matmul A: y chunks = Wcat1 col blocks ----
                ys = []
                for m in range(5):
                    m0 = m * 128
                    mw = min(128, 576 - m0)
                    yt = ypool.tile([mw, w], F32, name=f"y{m}")
                    for i in range(6):
                        nc.tensor.matmul(yt[:, :], w1_sb[i][:, m0:m0 + mw],
                                         xt_sb[k][i][:, :],
                                         start=(i == 0), stop=(i == 5))
                    ys.append(yt)

                zc = [zpool.tile([ZP[j], 480], BF16, name=f"zc{j}")
                      for j in range(5)]
                z_tiles[k] = zc

                # views [p, b, l, t]
                def v(ap, p0, p1):
                    return ap[p0:p1, 0:w].rearrange(
                        "p (b l t) -> p b l t", b=2, t=T)

                # ---- conv_t (main branch): center tap split over t-ranges
                # with pre-folded edge biases (ACT), then two accumulating
                # shifted taps (DVE) ----
                for (ysrc, yp0, yp1, zdst, cw0, cw1, cw2, cs2, cf0,
                     cf15, rows) in (
                        (ys[0], 0, 128, zc[0], 0, 1, 2, 3, 4, 5, 128),
                        (ys[1], 0, 64, zc[1], 6, 7, 8, 9, 10, 11, 64)):
                    yv = v(ysrc, yp0, yp1)
                    zv = v(zdst, yp0, yp1)
                    # full-range center (interior bias), then t-edge bias
                    # corrections on size-1 t-slices (both <=3D after opt)
                    nc.scalar.activation(
                        zv, yv, AF.Identity,
                        bias=col(cs2, rows), scale=col(cw1, rows))
                    nc.scalar.activation(
                        zv[:, :, :, 0:1], zv[:, :, :, 0:1], AF.Identity,
                        bias=col(cf0, rows), scale=1.0)
                    nc.scalar.activation(
                        zv[:, :, :, T - 1:T], zv[:, :, :, T - 1:T], AF.Identity,
                        bias=col(cf15, rows), scale=1.0)
                    # shifted taps per clip (3D APs); grouped left-left /
                    # right-right so chained RAW deps are 2 ops apart
                    for b in (0, 1):
                        nc.vector.scalar_tensor_tensor(
                            out=zv[:, b, :, 1:], in0=yv[:, b, :, :T - 1],
                            scalar=col(cw0, rows), in1=zv[:, b, :, 1:],
                            op0=OP.mult, op1=OP.add)
                    for b in (0, 1):
                        nc.vector.scalar_tensor_tensor(
                            out=zv[:, b, :, :T - 1], in0=yv[:, b, :, 1:],
                            scalar=col(cw2, rows), in1=zv[:, b, :, :T - 1],
                            op0=OP.mult, op1=OP.add)

                # ---- quickGELU branch (ACT) ----
                nc.scalar.activation(zc[1][64:128, :w], ys[1][64:128, :],
                                     AF.Silu, bias=colr(64, 128, 12),
                                     scale=1.702)
                nc.scalar.activation(zc[2][0:128, :w], ys[2][0:128, :],
                                     AF.Silu, bias=colr(0, 128, 13),
                                     scale=1.702)

                # ---- temporal diff -> padded buffers (h-row k) ----
                lp0 = (k + 1) * PADW + 1
                yo_a = zpool.tile([128, 480], BF16, name="yo_a")
                yo_b = zpool.tile([64, 480], BF16, name="yo_b")
                nc.scalar.activation(yo_a[:, :w], ys[3][:, :], AF.Copy)
                nc.scalar.activation(yo_b[:, :w], ys[4][:, :], AF.Copy)
                # chunk A (ca 0..128): [p, b, l, t] -> ppad_a[p, b, lpad, t]
                yv = v(yo_a, 0, 128)[:, :, loff:, :]
                pv = pav[:, :, lp0:lp0 + HGRID, :]
                for b in (0, 1):
                    nc.vector.scalar_tensor_tensor(
                        out=pv[:, b, :, 1:], in0=yv[:, b, :, 1:],
                        scalar=col(14, 128), in1=yv[:, b, :, :T - 1],
                        op0=OP.add, op1=OP.subtract)
                nc.scalar.activation(
                    pv[:, :, :, 0:1], yv[:, :, :, 0:1], AF.Identity,
                    bias=col(14, 128), scale=0.0)
                # chunk B: pack the two clips across partition halves
                # (contiguous b-blocks thanks to the b-major token order)
                yb2 = zpool.tile([128, 224], BF16, name="yb2")
                yo_b4 = v(yo_b, 0, 64)[:, :, loff:, :]
                nc.sync.dma_start(
                    out=yb2[0:64, :].rearrange("p (o l t) -> p o l t",
                                               o=1, t=T),
                    in_=yo_b4[:, 0:1])
                nc.sync.dma_start(
                    out=yb2[64:128, :].rearrange("p (o l t) -> p o l t",
                                                 o=1, t=T),
                    in_=yo_b4[:, 1:2])
                yv2 = yb2[0:128, :].rearrange("p (l t) -> p l t", t=T)
                pv2 = pbv[:, lp0:lp0 + HGRID, :]
                nc.vector.scalar_tensor_tensor(
                    out=pv2[:, :, 1:], in0=yv2[:, :, 1:],
                    scalar=col(37, 128), in1=yv2[:, :, :T - 1],
                    op0=OP.add, op1=OP.subtract)
                nc.scalar.activation(
                    pv2[:, :, 0:1], yv2[:, :, 0:1], AF.Identity,
                    bias=col(37, 128), scale=0.0)

                # ---- aux rows ----
                nc.gpsimd.memset(zc[4][64:66, :w], 1.0)
                if k == 0:
                    # zero aux2 + the offset-branch z rows on the CLS cols
                    cls4 = zc[4][64:66, 0:w].rearrange(
                        "p (b l t) -> p b l t", b=2, t=T)[:, :, 0:1, :]
                    nc.vector.tensor_scalar(
                        out=cls4, in0=cls4,
                        scalar1=colr(64, 66, 36), scalar2=None, op0=OP.mult)
                    nc.gpsimd.memset(zc[3][0:128, 0:w].rearrange(
                        "p (b l t) -> p b l t", b=2, t=T)[:, :, 0:1, :], 0.0)
                    nc.gpsimd.memset(zc[4][0:64, 0:w].rearrange(
                        "p (b l t) -> p b l t", b=2, t=T)[:, :, 0:1, :], 0.0)

            def emit_spconv_B_out(j):
                nl = TILES[j][1]
                w = nl * NL
                c0 = TILE_C0[j]
                loff = 1 if j == 0 else 0
                lp0 = (j + 1) * PADW + 1
                zc = z_tiles[j]
                # ---- spatial conv: 9 shifted taps from the padded buffers
                # (chunk A full-partition; chunk B b-packed, then unpacked
                # into the z layout via two contiguous DMAs) ----
                zv = zc[3][0:128, 0:w].rearrange(
                    "p (b l t) -> p b l t", b=2, t=T)[:, :, loff:, :]
                qb2 = zpool.tile([128, 224], BF16, name="qb2")
                qv = qb2[0:128, :].rearrange("p (l t) -> p l t", t=T)
                # interleave the two accumulation chains so consecutive DVE
                # ops never have an adjacent RAW dependency
                for t_i, (dh, dw) in enumerate(TAPS):
                    sl = lp0 + dh * PADW + dw
                    pv = pav[:, :, sl:sl + HGRID, :]
                    pv2 = pbv[:, sl:sl + HGRID, :]
                    if t_i == 0:
                        nc.scalar.activation(
                            zv, pv, AF.Identity,
                            bias=col(34, 128), scale=col(16 + t_i, 128))
                        nc.scalar.activation(
                            qv, pv2, AF.Identity,
                            bias=col(47, 128), scale=col(38 + t_i, 128))
                    else:
                        nc.vector.scalar_tensor_tensor(
                            out=zv, in0=pv, scalar=col(16 + t_i, 128),
                            in1=zv, op0=OP.mult, op1=OP.add)
                        nc.vector.scalar_tensor_tensor(
                            out=qv, in0=pv2, scalar=col(38 + t_i, 128),
                            in1=qv, op0=OP.mult, op1=OP.add)
                zb = zc[4][0:64, 0:w].rearrange(
                    "p (b l t) -> p b l t", b=2, t=T)[:, :, loff:, :]
                qb4 = qb2.rearrange("p (o l t) -> p o l t", o=1, t=T)
                nc.sync.dma_start(out=zb[:, 0:1], in_=qb4[0:64])
                nc.sync.dma_start(out=zb[:, 1:2], in_=qb4[64:128])

                # ---- matmul B + eviction + store ----
                for m in range(6):
                    m0 = m * 128
                    ot = opool.tile([128, w], F32, name="ops")
                    for kk in range(5):
                        nc.tensor.matmul(ot[:, :], w2_sb[kk][:, m0:m0 + 128],
                                         zc[kk][:, :w],
                                         start=(kk == 0), stop=(kk == 4))
                    osb = opool_sb.tile([128, w], BF16, name="osb")
                    nc.scalar.activation(osb[:, :], ot[:, :], AF.Copy)
                    nc.sync.dma_start(out=out_d[m0:m0 + 128, c0:c0 + w],
                                      in_=osb[:, :])

            for k in range(3, 14):
                load_xt(k)
            for k in range(14):
                emit_A_and_middle(k)
                if k >= 2:
                    emit_spconv_B_out(k - 2)
            emit_spconv_B_out(12)
            emit_spconv_B_out(13)

    nc.compile()
    return nc


_cached = {}


def _get_kernel():
    if "nc" not in _cached:
        _cached["nc"] = build_kernel()
    return _cached["nc"]


def _host_xt(x):
    """x (L, 256, C) f32 -> (8, C, NCOLS) bf16, per-tile b-major token order."""
    bf = ml_dtypes.bfloat16
    out = np.empty((NCORES, C, NCOLS), bf)
    x5 = x.reshape(L, NCORES, 2, T, C)
    for k, (l0, nl) in enumerate(TILES):
        blk = x5[l0:l0 + nl]                      # (nl, 8, 2, T, C)
        blk = blk.transpose(1, 4, 2, 0, 3)        # (8, C, 2, nl, T)
        out[:, :, TILE_C0[k]:TILE_C0[k + 1]] = (
            blk.reshape(NCORES, C, nl * NL).astype(bf))
    return out


def _host_out(outT):
    """outT (8, C, NCOLS) -> out (L, 256, C) f32."""
    out = np.empty((L, NCORES * NL, C), np.float32)
    for k, (l0, nl) in enumerate(TILES):
        blk = outT[:, :, TILE_C0[k]:TILE_C0[k + 1]].astype(np.float32)
        blk = blk.reshape(NCORES, C, 2, nl, T)    # (8, C, 2, nl, T)
        blk = blk.transpose(3, 0, 2, 4, 1)        # (nl, 8, 2, T, C)
        out[l0:l0 + nl] = blk.reshape(nl, NCORES * NL, C)
    return out


def kernel(x, T, fc1_w, fc1_b, conv_w, conv_b, fc2_w, fc2_b,
           off_fc1_w, off_fc1_b, off_conv_w, off_conv_b, off_fc2_w, off_fc2_b,
           mlp_in_w, mlp_in_b, mlp_out_w, mlp_out_b):
    bf = ml_dtypes.bfloat16
    x = np.asarray(x, np.float32)
    to_np = lambda a: np.asarray(a, np.float32)
    (fc1_w, fc1_b, conv_w, conv_b, fc2_w, fc2_b, off_fc1_w, off_fc1_b,
     off_conv_w, off_conv_b, off_fc2_w, off_fc2_b, mlp_in_w, mlp_in_b,
     mlp_out_w, mlp_out_b) = map(to_np, (
        fc1_w, fc1_b, conv_w, conv_b, fc2_w, fc2_b, off_fc1_w, off_fc1_b,
        off_conv_w, off_conv_b, off_fc2_w, off_fc2_b, mlp_in_w, mlp_in_b,
        mlp_out_w, mlp_out_b))

    xt = _host_xt(x)

    w1 = np.concatenate([fc1_w, mlp_in_w, off_fc1_w], axis=1).astype(bf)
    w2 = np.concatenate([
        fc2_w,
        mlp_out_w / 1.702,
        off_fc2_w,
        (fc2_b + mlp_out_b)[None, :],
        off_fc2_b[None, :],
    ], axis=0).astype(bf)
    scal = _pack_scalars(conv_w, conv_b, fc1_b, mlp_in_b, off_fc1_b,
                         off_conv_w, off_conv_b)

    nc = _get_kernel()
    in_maps = [{"xt": xt[i], "w1": w1, "w2": w2, "scal": scal}
               for i in range(NCORES)]
    res = bass_utils.run_bass_kernel_spmd(nc, in_maps,
                                          core_ids=list(range(NCORES)))
    _cached["last_result"] = res

    outT = np.stack([np.asarray(res.results[i]["out"]) for i in range(NCORES)])
    return np.ascontiguousarray(_host_out(outT))
